# revision 18
# baseline (speedup 1.0000x reference)
"""Trainium2 Bass kernel for ContractExpand (segment_reduce).

For each scale r in (1,2,4,10,25): segment-sum groups of r consecutive rows,
relu(Linear_r)/r, broadcast back to rows, concat along rows.

Strategy: pure data parallel over 8 cores (row-sharded, 100-row aligned).
Per core, per 200-row chunk:
  mm#1: tmpT[k, g] = x_chunk[:, k].T @ seg  (seg = 0/1 segment matrix, all
        scales at once; also yields the r=1 transpose). fp32r, N=378.
  copy: PSUM -> SBUF tmpT buffers laid out so that r1/r2 group columns are
        contiguous across chunks (full 128-wide M tiles downstream).
  mm#2: h[g, :] = relu(tmpT[:, gtile].T @ WtExt_r) where WtExt_r host-prepped
        as [W_r.T | b_r] / r with an all-ones row appended to tmpT (bias via
        matmul). 3 accumulating k-slices, fp32r, N=300.
  out:  DMA h to DRAM with a step-0 broadcast AP: each group row is written r
        consecutive times -> fully contiguous HBM writes.
"""

import os
import sys

import numpy as np

if "/opt/trn_rl_repo" not in sys.path:
    sys.path.insert(0, "/opt/trn_rl_repo")

from contextlib import ExitStack

import concourse.bass as bass
import concourse.tile as tile
from concourse import bacc, mybir

DIM = 300
SCALES = (1, 2, 4, 10, 25)
N_TOTAL = 100000
N_CORES = 8
R_PER_CORE = N_TOTAL // N_CORES  # 12500
CHUNK = 200  # rows per mm#1 chunk (two 100-row subtiles)
SC_CHUNKS = 8  # chunks per superchunk (tmpT buffer granularity)

# column offsets of each scale inside a 378-wide per-chunk mm#1 output block
SEG_COLS = 378  # 200 + 100 + 50 + 20 + 8
SEG_OFF = {1: 0, 2: 200, 4: 300, 10: 350, 25: 370}
REST_W = 78  # width of the r4/r10/r25 region per chunk (50 + 20 + 8)
REST_OFF = {4: 0, 10: 50, 25: 70}

# k slices over the extended contraction dim (300 dims + 1 bias row)
KSLICES = [(0, 128), (128, 256), (256, 301)]  # sizes 128, 128, 45
F32 = mybir.dt.float32
F32R = mybir.dt.float32r
DT = mybir.dt.float16  # matmul datapath dtype (PE streams 16-bit at 1 cy/row)
NPDT = np.float16


def _chunk_list(rows):
    """rows -> list of chunk sizes (200s then possibly one 100)."""
    assert rows % 100 == 0
    chunks = [CHUNK] * (rows // CHUNK)
    if rows % CHUNK:
        chunks.append(100)
    return chunks


def _superchunks(chunks):
    """Group chunk sizes into superchunks of <= SC_CHUNKS chunks."""
    return [chunks[i : i + SC_CHUNKS] for i in range(0, len(chunks), SC_CHUNKS)]


def _emit(ctx, tc, x_ap, wt_ap, seg_ap, ones_ap, out_ap, rows):
    nc = tc.nc

    singles = ctx.enter_context(tc.tile_pool(name="singles", bufs=1))
    xpool = ctx.enter_context(tc.tile_pool(name="xb", bufs=2))
    hpool = ctx.enter_context(tc.tile_pool(name="h", bufs=4))
    p1pool = ctx.enter_context(tc.tile_pool(name="p1", bufs=2, space="PSUM"))
    p2pool = ctx.enter_context(tc.tile_pool(name="p2", bufs=2, space="PSUM"))

    # --- constants ---
    seg_sb = singles.tile([100, 2, SEG_COLS], DT, tag="seg")
    nc.sync.dma_start(out=seg_sb[:], in_=seg_ap[:])

    wt_sb = []  # [scale][kslice] -> SBUF tile [ksz, 300]
    for i in range(len(SCALES)):
        per_s = []
        for s, (k0, k1) in enumerate(KSLICES):
            t = singles.tile([k1 - k0, DIM], DT, tag=f"wt{i}_{s}")
            nc.sync.dma_start(out=t[:], in_=wt_ap[i, k0:k1, :])
            per_s.append(t)
        wt_sb.append(per_s)

    chunks = _chunk_list(rows)
    scs = _superchunks(chunks)
    max_cols = SC_CHUNKS * SEG_COLS  # tmpT column capacity per buffer

    # two persistent tmpT buffers (ping-pong across superchunks), 3 k-slices
    # each; slice 2 has 44 data rows + an all-ones row at partition 44 (bias).
    tmpT = []
    for b in range(2):
        slices = []
        for s, (k0, k1) in enumerate(KSLICES):
            t = singles.tile([k1 - k0, max_cols], DT, tag=f"tmpT{b}_{s}")
            slices.append(t)
        nc.sync.dma_start(out=slices[2][44:45, :], in_=ones_ap[:])
        tmpT.append(slices)

    row0 = 0  # global row index of current superchunk
    chunk0 = 0  # global chunk index of current superchunk
    for sci, sc in enumerate(scs):
        buf = tmpT[sci % 2]
        sc_rows = sum(sc)
        nsub = sc_rows // 100
        # per-scale region layout within this superchunk's tmpT columns:
        # each scale's group-columns are contiguous across the SC's chunks
        widths = [sc_rows // r for r in SCALES]
        bases = [0]
        for w in widths[:-1]:
            bases.append(bases[-1] + w)

        # --- load x for the whole superchunk (one big DMA) ---
        xb = xpool.tile([100, 2 * SC_CHUNKS, DIM], DT, tag="xb")
        src = x_ap[row0 : row0 + sc_rows, :].rearrange("(c p) d -> p c d", p=100)
        nc.sync.dma_start(out=xb[:, :nsub, :], in_=src)

        # --- mm#1 + copies per chunk ---
        for b, csz in enumerate(sc):
            p1 = []
            for s, (k0, k1) in enumerate(KSLICES):
                ksz = min(k1, DIM) - k0  # x has only 300 cols (no bias row)
                pt = p1pool.tile([ksz, SEG_COLS], F32, tag=f"p1_{s}")
                nsubc = csz // 100
                for t in range(nsubc):
                    nc.tensor.matmul(
                        pt[:],
                        xb[:, 2 * b + t, k0 : k0 + ksz],
                        seg_sb[:, t, :],
                        start=(t == 0),
                        stop=(t == nsubc - 1),
                    )
                p1.append(pt)
            # copies PSUM -> SBUF (per-scale region-major). b-th chunk of SC.
            for s in range(3):
                ksz = p1[s].shape[0]
                for si, r in enumerate(SCALES):
                    w = csz // r
                    dst0 = bases[si] + (CHUNK // r) * b
                    eng = nc.vector if r <= 2 else nc.any
                    eng.tensor_copy(
                        out=buf[s][:ksz, dst0 : dst0 + w],
                        in_=p1[s][:, SEG_OFF[r] : SEG_OFF[r] + w],
                    )

        # --- mm#2 + relu + expand-store per M tile ---
        for si, r in enumerate(SCALES):
            for c0 in range(0, widths[si], 128):
                M = min(128, widths[si] - c0)
                g0 = chunk0 * (CHUNK // r) + c0
                psum2 = p2pool.tile([128, DIM], F32, tag="p2")
                for s in range(3):
                    nc.tensor.matmul(
                        psum2[:M, :],
                        buf[s][:, bases[si] + c0 : bases[si] + c0 + M],
                        wt_sb[si][s][:],
                        start=(s == 0),
                        stop=(s == 2),
                    )
                h = hpool.tile([128, DIM], F32, tag="h")
                nc.scalar.activation(
                    out=h[:M, :],
                    in_=psum2[:M, :],
                    func=mybir.ActivationFunctionType.Relu,
                )
                orow = si * rows + g0 * r
                if r == 1:
                    nc.sync.dma_start(out=out_ap[orow : orow + M, :], in_=h[:M, :])
                else:
                    dst = out_ap[orow : orow + M * r, :].rearrange(
                        "(g q) d -> g q d", q=r
                    )
                    src_b = h[:M, :].unsqueeze(1).broadcast_to([M, r, DIM])
                    nc.sync.dma_start(out=dst, in_=src_b)

        row0 += sc_rows
        chunk0 += len(sc)


def build_nc(rows=R_PER_CORE):
    nc = bacc.Bacc("TRN2", target_bir_lowering=False)
    x = nc.declare_dram_parameter("x", [rows, DIM], DT, isOutput=False)
    wt = nc.declare_dram_parameter("wt", [len(SCALES), DIM + 1, DIM], DT, isOutput=False)
    seg = nc.declare_dram_parameter("seg", [100, 2, SEG_COLS], DT, isOutput=False)
    ones = nc.declare_dram_parameter(
        "ones", [1, SC_CHUNKS * SEG_COLS], DT, isOutput=False
    )
    out = nc.declare_dram_parameter("out", [len(SCALES) * rows, DIM], F32, isOutput=True)
    with tile.TileContext(nc) as tc:
        with ExitStack() as ctx:
            _emit(ctx, tc, x.ap(), wt.ap(), seg.ap(), ones.ap(), out.ap(), rows)
    nc.compile()
    return nc


def round_fp32r(a):
    """Round fp32 -> fp32r (1s + 8e + 11m; low 12 mantissa bits zero)."""
    b = np.ascontiguousarray(a, np.float32).view(np.uint32)
    r = ((b.astype(np.uint64) + 0x800) & 0xFFFFF000).astype(np.uint32)
    return r.view(np.float32)


def make_seg():
    """[100, 2, 378] fp32: seg[p, t, off_r + (t*100+p)//r] = 1."""
    seg = np.zeros((100, 2, SEG_COLS), np.float32)
    for t in range(2):
        for p in range(100):
            row = t * 100 + p
            for r in SCALES:
                seg[p, t, SEG_OFF[r] + row // r] = 1.0
    return seg


def make_wt(Ws, bs):
    """[5, 301, 300] fp32: [W_r.T ; b_r] / r."""
    wt = np.empty((len(SCALES), DIM + 1, DIM), np.float32)
    for i, r in enumerate(SCALES):
        wt[i, :DIM, :] = np.asarray(Ws[i], np.float32).T / r
        wt[i, DIM, :] = np.asarray(bs[i], np.float32) / r
    return wt


_NC_CACHE = {}


def _get_nc(rows):
    if rows not in _NC_CACHE:
        _NC_CACHE[rows] = build_nc(rows)
    return _NC_CACHE[rows]


def run_cores(inputs_c_e, Ws, bs, trace=False, **kw):
    """Shard, run on the 8 NeuronCores, gather. Returns (full_out, results)."""
    from concourse.bass_utils import run_bass_kernel_spmd

    x = np.ascontiguousarray(np.asarray(inputs_c_e, np.float32)).astype(NPDT)
    n = x.shape[0]
    assert n % (N_CORES * 100) == 0
    rows = n // N_CORES
    wt = make_wt(Ws, bs).astype(NPDT)
    seg = make_seg().astype(NPDT)
    nc = _get_nc(rows)
    ones = np.ones((1, SC_CHUNKS * SEG_COLS), NPDT)
    in_maps = [
        {"x": x[c * rows : (c + 1) * rows], "wt": wt, "seg": seg, "ones": ones}
        for c in range(N_CORES)
    ]
    res = run_bass_kernel_spmd(nc, in_maps, list(range(N_CORES)), trace=trace, **kw)
    full = np.empty((len(SCALES) * n, DIM), np.float32)
    for si in range(len(SCALES)):
        for c in range(N_CORES):
            full[si * n + c * rows : si * n + (c + 1) * rows] = res.results[c]["out"][
                si * rows : (si + 1) * rows
            ]
    return full, res


def kernel(inputs_c_e, Ws, bs):
    full, _ = run_cores(inputs_c_e, Ws, bs)
    return full


# revision 22
# speedup vs baseline: 1.5183x; 1.5183x over previous
"""Trainium2 Bass kernel for ContractExpand (segment_reduce).

For each scale r in (1,2,4,10,25): segment-sum groups of r consecutive rows,
relu(Linear_r)/r, broadcast back to rows, concat all scales along rows.

Strategy: pure data parallel over 8 cores (row-sharded; shards padded to
12800 rows so every core runs an identical 4x3200-row program).

Per core, per 3200-row superchunk (all matmul data in fp16):
  transpose: xT[k, n] via hardware DMA-transpose of the padded x (384 cols =
        3 k-slices of 128). Host writes 1.0 into column 300, so the
        transpose lands an all-ones row at k-slice-2 row 44 -> the bias
        folds into the GEMM (host pre-scales wt bias row by r/r^2).
  reduce: DVE strided reduce_sum builds the r2/r4/r10/r25 segment sums from
        xT (r4 and r10 reuse the r2 sums). The ones row reduces to r, which
        the host-side b/r^2 bias row compensates.
  mm#2: h[g, :] = relu(lhsT.T @ WtExt_r): lhsT = xT (r=1) or the reduced
        region, 3 accumulating k-slices (128+128+45incl bias), N=300.
  out:  DMA h to DRAM with a step-0 broadcast AP: each group row is written
        r consecutive times -> fully contiguous HBM writes.
"""

import sys

import numpy as np

if "/opt/trn_rl_repo" not in sys.path:
    sys.path.insert(0, "/opt/trn_rl_repo")

from contextlib import ExitStack

import concourse.tile as tile
from concourse import bacc, mybir

DIM = 300
KPAD = 384  # padded x cols: 300 dims + ones col + zeros (3 slices of 128)
SCALES = (1, 2, 4, 10, 25)
N_TOTAL = 100000
N_CORES = 8
R_VALID = N_TOTAL // N_CORES  # 12500 real rows per core
R_PAD = 12800  # padded to 4 x 3200
SC_ROWS = 3200
# k slices over the extended contraction dim (300 dims + 1 bias row)
KSLICES = [(0, 128), (128, 256), (256, 301)]  # sizes 128, 128, 45
F32 = mybir.dt.float32
DT = mybir.dt.float16
NPDT = np.float16
AXX = mybir.AxisListType.X


def _superchunks(rows):
    out = []
    while rows > 0:
        sc = min(SC_ROWS, rows)
        assert sc % 400 == 0
        out.append(sc)
        rows -= sc
    return out


def _emit(ctx, tc, x_ap, wt_ap, out_ap, rows):
    nc = tc.nc

    singles = ctx.enter_context(tc.tile_pool(name="singles", bufs=1))
    hpool = ctx.enter_context(tc.tile_pool(name="h", bufs=6))
    p2pool = ctx.enter_context(tc.tile_pool(name="p2", bufs=6, space="PSUM"))

    wt_sb = []  # [scale][kslice] -> SBUF tile [ksz, 300] fp16
    for i in range(len(SCALES)):
        per_s = []
        for s, (k0, k1) in enumerate(KSLICES):
            t = singles.tile([k1 - k0, DIM], DT, tag=f"wt{i}_{s}")
            nc.sync.dma_start(out=t[:], in_=wt_ap[i, k0:k1, :])
            per_s.append(t)
        wt_sb.append(per_s)

    scs = _superchunks(rows)
    max_red = sum(SC_ROWS // r for r in SCALES[1:])  # 2848

    # double-buffered xT (transposed x) and tmpT (reduced sums) tiles
    xT = [
        [singles.tile([128, SC_ROWS], DT, tag=f"xT{b}_{s}", name=f"xT{b}_{s}") for s in range(3)]
        for b in range(2)
    ]
    tmpT = [
        [
            singles.tile([k1 - k0, max_red], DT, tag=f"tm{b}_{s}", name=f"tm{b}_{s}")
            for s, (k0, k1) in enumerate(KSLICES)
        ]
        for b in range(2)
    ]

    row0 = 0
    for sci, sc_rows in enumerate(scs):
        xb = xT[sci % 2]
        tb = tmpT[sci % 2]
        widths = [sc_rows // r for r in SCALES]  # r1 width unused below
        # region base offsets of r2/r4/r10/r25 inside tmpT
        rb = [0, widths[1], widths[1] + widths[2], widths[1] + widths[2] + widths[3]]

        # --- hardware DMA transpose: x[row0:+sc, 128s:128s+128] -> [128, sc]
        for s in range(3):
            nc.sync.dma_start_transpose(
                out=xb[s][:, :sc_rows],
                in_=x_ap[row0 : row0 + sc_rows, 128 * s : 128 * (s + 1)],
            )

        # --- DVE segment reductions (fp16). ones row reduces to r. ---
        lp = nc.allow_low_precision(reason="fp16 segment sums feed fp16 matmul")
        lp.__enter__()
        for s, (k0, k1) in enumerate(KSLICES):
            ksz = k1 - k0
            src = xb[s][:ksz, :sc_rows]
            r2 = tb[s][:ksz, rb[0] : rb[0] + widths[1]]
            nc.vector.reduce_sum(
                out=r2, in_=src.rearrange("p (g r) -> p g r", r=2), axis=AXX
            )
            nc.vector.reduce_sum(
                out=tb[s][:ksz, rb[1] : rb[1] + widths[2]],
                in_=r2.rearrange("p (g r) -> p g r", r=2),
                axis=AXX,
            )
            nc.vector.reduce_sum(
                out=tb[s][:ksz, rb[2] : rb[2] + widths[3]],
                in_=r2.rearrange("p (g r) -> p g r", r=5),
                axis=AXX,
            )
            nc.vector.reduce_sum(
                out=tb[s][:ksz, rb[3] : rb[3] + widths[4]],
                in_=src.rearrange("p (g r) -> p g r", r=25),
                axis=AXX,
            )
        lp.__exit__(None, None, None)

        # --- mm#2 + relu + expand-store per M tile ---
        for si, r in enumerate(SCALES):
            width = sc_rows // r
            base = 0 if r == 1 else rb[si - 1]
            srcs = xb if r == 1 else tb
            for c0 in range(0, width, 128):
                M = min(128, width - c0)
                g0 = row0 // r + c0
                psum2 = p2pool.tile([128, DIM], F32, tag="p2")
                for s, (k0, k1) in enumerate(KSLICES):
                    ksz = k1 - k0
                    nc.tensor.matmul(
                        psum2[:M, :],
                        srcs[s][:ksz, base + c0 : base + c0 + M],
                        wt_sb[si][s][:],
                        start=(s == 0),
                        stop=(s == 2),
                    )
                h = hpool.tile([128, DIM], F32, tag="h")
                nc.scalar.activation(
                    out=h[:M, :],
                    in_=psum2[:M, :],
                    func=mybir.ActivationFunctionType.Relu,
                )
                orow = si * rows + g0 * r
                if r == 1:
                    nc.sync.dma_start(out=out_ap[orow : orow + M, :], in_=h[:M, :])
                else:
                    dst = out_ap[orow : orow + M * r, :].rearrange(
                        "(g q) d -> g q d", q=r
                    )
                    src_b = h[:M, :].unsqueeze(1).broadcast_to([M, r, DIM])
                    nc.sync.dma_start(out=dst, in_=src_b)

        row0 += sc_rows


def build_nc(rows=R_PAD):
    nc = bacc.Bacc("TRN2", target_bir_lowering=False)
    x = nc.declare_dram_parameter("x", [rows, KPAD], DT, isOutput=False)
    wt = nc.declare_dram_parameter(
        "wt", [len(SCALES), DIM + 1, DIM], DT, isOutput=False
    )
    out = nc.declare_dram_parameter(
        "out", [len(SCALES) * rows, DIM], F32, isOutput=True
    )
    with tile.TileContext(nc) as tc:
        with ExitStack() as ctx:
            _emit(ctx, tc, x.ap(), wt.ap(), out.ap(), rows)
    nc.compile()
    return nc


def make_wt(Ws, bs):
    """[5, 301, 300]: [W_r.T / r ; b_r / r^2] (ones row reduces to r)."""
    wt = np.empty((len(SCALES), DIM + 1, DIM), np.float32)
    for i, r in enumerate(SCALES):
        wt[i, :DIM, :] = np.asarray(Ws[i], np.float32).T / r
        wt[i, DIM, :] = np.asarray(bs[i], np.float32) / (r * r)
    return wt


def pad_x(x_shard, rows_pad=R_PAD):
    """[n, 300] fp32 -> [rows_pad, 384] fp16 with ones in col 300."""
    xp = np.zeros((rows_pad, KPAD), NPDT)
    xp[: len(x_shard), :DIM] = x_shard.astype(NPDT)
    xp[:, DIM] = 1.0
    return xp


_NC_CACHE = {}


def _get_nc(rows):
    if rows not in _NC_CACHE:
        _NC_CACHE[rows] = build_nc(rows)
    return _NC_CACHE[rows]


def run_cores(inputs_c_e, Ws, bs, trace=False, **kw):
    """Shard, run on the 8 NeuronCores, gather. Returns (full_out, results)."""
    from concourse.bass_utils import run_bass_kernel_spmd

    x = np.ascontiguousarray(np.asarray(inputs_c_e, np.float32))
    n = x.shape[0]
    assert n == N_TOTAL
    wt = make_wt(Ws, bs).astype(NPDT)
    nc = _get_nc(R_PAD)
    in_maps = [
        {"x": pad_x(x[c * R_VALID : (c + 1) * R_VALID]), "wt": wt}
        for c in range(N_CORES)
    ]
    res = run_bass_kernel_spmd(nc, in_maps, list(range(N_CORES)), trace=trace, **kw)
    full = np.empty((len(SCALES) * n, DIM), np.float32)
    for si in range(len(SCALES)):
        for c in range(N_CORES):
            full[si * n + c * R_VALID : si * n + (c + 1) * R_VALID] = res.results[c][
                "out"
            ][si * R_PAD : si * R_PAD + R_VALID]
    return full, res


def kernel(inputs_c_e, Ws, bs):
    full, _ = run_cores(inputs_c_e, Ws, bs)
    return full


# revision 27
# speedup vs baseline: 1.5919x; 1.0484x over previous
"""Trainium2 Bass kernel for ContractExpand (segment_reduce).

For each scale r in (1,2,4,10,25): segment-sum groups of r consecutive rows,
relu(Linear_r)/r, broadcast back to rows, concat all scales along rows.

Strategy: pure data parallel over 8 NeuronCores (row-sharded, 12500 rows per
core). The host passes x TRANSPOSED ([301, 12500] fp16, row 300 = ones), so
the kernel's input loads are large contiguous DMAs and the contraction dim
is already on partitions.

Per core, per superchunk of up to 3200 rows (matmul data in fp16):
  load:   xT k-slices [128|128|45, sc] straight from DRAM.
  reduce: DVE strided reduce_sum builds r2/r4/r10/r25 segment sums from xT
          (r4, r10 reuse the r2 sums). The ones row reduces to r, which the
          host-side bias row b/r^2 compensates (bias folds into the GEMM).
  mm:     h[g, :] = relu(lhsT.T @ WtExt_r), lhsT = xT (r=1) or a reduced
          region; 3 accumulating k-slices (128+128+45 incl bias), N=300.
  store:  relu result is replicated rep_r times along SBUF free dim, then
          DMA'd with a step-0 broadcast AP -> fully contiguous HBM writes
          with rep_r*1200B descriptors.
"""

import sys

import numpy as np

if "/opt/trn_rl_repo" not in sys.path:
    sys.path.insert(0, "/opt/trn_rl_repo")

from contextlib import ExitStack

import concourse.tile as tile
from concourse import bacc, mybir

DIM = 300
KEXT = 301  # 300 dims + ones row
SCALES = (1, 2, 4, 10, 25)
REP = {1: 1, 2: 2, 4: 4, 10: 5, 25: 5}  # SBUF replication factor per scale
N_TOTAL = 100000
N_CORES = 8
R_CORE = N_TOTAL // N_CORES  # 12500
SC_ROWS = 3200
KSLICES = [(0, 128), (128, 256), (256, 301)]  # sizes 128, 128, 45
F32 = mybir.dt.float32
DT = mybir.dt.float16
NPDT = np.float16
AXX = mybir.AxisListType.X


def _superchunks(rows):
    out = []
    while rows > 0:
        sc = min(SC_ROWS, rows)
        assert sc % 100 == 0
        out.append(sc)
        rows -= sc
    return out


def _emit(ctx, tc, xt_ap, wt_ap, out_ap, rows):
    nc = tc.nc

    singles = ctx.enter_context(tc.tile_pool(name="singles", bufs=1))
    hpool = ctx.enter_context(tc.tile_pool(name="h", bufs=4))
    p2pool = ctx.enter_context(tc.tile_pool(name="p2", bufs=6, space="PSUM"))

    wt_sb = []  # [scale][kslice] -> SBUF tile [ksz, 300] fp16
    for i in range(len(SCALES)):
        per_s = []
        for s, (k0, k1) in enumerate(KSLICES):
            t = singles.tile([k1 - k0, DIM], DT, tag=f"wt{i}_{s}")
            nc.sync.dma_start(out=t[:], in_=wt_ap[i, k0:k1, :])
            per_s.append(t)
        wt_sb.append(per_s)

    scs = _superchunks(rows)
    max_red = sum((SC_ROWS // r + 63) & ~63 for r in SCALES[1:])  # 2880

    # double-buffered xT (k-sliced transposed x) and tmpT (reduced sums)
    xT = [
        [
            singles.tile([k1 - k0, SC_ROWS], DT, tag=f"xT{b}_{s}", name=f"xT{b}_{s}")
            for s, (k0, k1) in enumerate(KSLICES)
        ]
        for b in range(2)
    ]
    tmpT = [
        [
            singles.tile([k1 - k0, max_red], DT, tag=f"tm{b}_{s}", name=f"tm{b}_{s}")
            for s, (k0, k1) in enumerate(KSLICES)
        ]
        for b in range(2)
    ]

    row0 = 0
    for sci, sc_rows in enumerate(scs):
        xb = xT[sci % 2]
        tb = tmpT[sci % 2]
        widths = [sc_rows // r for r in SCALES]
        # region bases padded to 64 fp16 elements (128B) so every matmul
        # lhsT base stays 4-byte aligned even for non-3200 superchunks
        rb, off = [], 0
        for w in widths[1:]:
            rb.append(off)
            off += (w + 63) & ~63

        # --- load xT k-slices (contiguous DMA from host-transposed x) ---
        for s, (k0, k1) in enumerate(KSLICES):
            nc.sync.dma_start(
                out=xb[s][: k1 - k0, :sc_rows],
                in_=xt_ap[k0:k1, row0 : row0 + sc_rows],
            )

        # --- DVE segment reductions (fp16). ones row reduces to r. ---
        lp = nc.allow_low_precision(reason="fp16 segment sums feed fp16 matmul")
        lp.__enter__()
        for s, (k0, k1) in enumerate(KSLICES):
            ksz = k1 - k0
            src = xb[s][:ksz, :sc_rows]
            r2 = tb[s][:ksz, rb[0] : rb[0] + widths[1]]
            nc.vector.reduce_sum(
                out=r2, in_=src.rearrange("p (g r) -> p g r", r=2), axis=AXX
            )
            nc.vector.reduce_sum(
                out=tb[s][:ksz, rb[1] : rb[1] + widths[2]],
                in_=r2.rearrange("p (g r) -> p g r", r=2),
                axis=AXX,
            )
            nc.vector.reduce_sum(
                out=tb[s][:ksz, rb[2] : rb[2] + widths[3]],
                in_=r2.rearrange("p (g r) -> p g r", r=5),
                axis=AXX,
            )
            nc.vector.reduce_sum(
                out=tb[s][:ksz, rb[3] : rb[3] + widths[4]],
                in_=src.rearrange("p (g r) -> p g r", r=25),
                axis=AXX,
            )
        lp.__exit__(None, None, None)

        # --- mm + relu + replicate + expand-store per M tile ---
        for si, r in enumerate(SCALES):
            width = sc_rows // r
            base = 0 if r == 1 else rb[si - 1]
            srcs = xb if r == 1 else tb
            rep = REP[r]
            for c0 in range(0, width, 128):
                M = min(128, width - c0)
                g0 = row0 // r + c0
                psum2 = p2pool.tile([128, DIM], F32, tag="p2")
                for s, (k0, k1) in enumerate(KSLICES):
                    ksz = k1 - k0
                    nc.tensor.matmul(
                        psum2[:M, :],
                        srcs[s][:ksz, base + c0 : base + c0 + M],
                        wt_sb[si][s][:],
                        start=(s == 0),
                        stop=(s == 2),
                    )
                h = hpool.tile([128, 5 * DIM], F32, tag="h")
                nc.scalar.activation(
                    out=h[:M, :DIM],
                    in_=psum2[:M, :],
                    func=mybir.ActivationFunctionType.Relu,
                )
                # replicate along free dim by doubling copies
                done = 1
                while done < rep:
                    cnt = min(done, rep - done)
                    nc.any.tensor_copy(
                        out=h[:M, done * DIM : (done + cnt) * DIM],
                        in_=h[:M, : cnt * DIM],
                    )
                    done += cnt
                orow = si * rows + g0 * r
                if r == 1:
                    nc.sync.dma_start(
                        out=out_ap[orow : orow + M, :], in_=h[:M, :DIM]
                    )
                else:
                    j = r // rep  # outer broadcast count
                    dst = out_ap[orow : orow + M * r, :].rearrange(
                        "(g j e) d -> g j (e d)", j=j, e=rep
                    )
                    src_b = (
                        h[:M, : rep * DIM]
                        .unsqueeze(1)
                        .broadcast_to([M, j, rep * DIM])
                    )
                    nc.sync.dma_start(out=dst, in_=src_b)

        row0 += sc_rows


def build_nc(rows=R_CORE):
    nc = bacc.Bacc("TRN2", target_bir_lowering=False)
    xt = nc.declare_dram_parameter("xt", [KEXT, rows], DT, isOutput=False)
    wt = nc.declare_dram_parameter(
        "wt", [len(SCALES), KEXT, DIM], DT, isOutput=False
    )
    out = nc.declare_dram_parameter(
        "out", [len(SCALES) * rows, DIM], F32, isOutput=True
    )
    with tile.TileContext(nc) as tc:
        with ExitStack() as ctx:
            _emit(ctx, tc, xt.ap(), wt.ap(), out.ap(), rows)
    nc.compile()
    return nc


def make_wt(Ws, bs):
    """[5, 301, 300]: [W_r.T / r ; b_r / r^2] (ones row reduces to r)."""
    wt = np.empty((len(SCALES), KEXT, DIM), np.float32)
    for i, r in enumerate(SCALES):
        wt[i, :DIM, :] = np.asarray(Ws[i], np.float32).T / r
        wt[i, DIM, :] = np.asarray(bs[i], np.float32) / (r * r)
    return wt


def make_xt(x_shard):
    """[n, 300] fp32 -> [301, n] fp16 with ones in row 300."""
    n = len(x_shard)
    xt = np.empty((KEXT, n), NPDT)
    xt[:DIM, :] = x_shard.astype(NPDT).T
    xt[DIM, :] = 1.0
    return np.ascontiguousarray(xt)


_NC_CACHE = {}


def _get_nc(rows):
    if rows not in _NC_CACHE:
        _NC_CACHE[rows] = build_nc(rows)
    return _NC_CACHE[rows]


def run_cores(inputs_c_e, Ws, bs, trace=False, **kw):
    """Shard, run on the 8 NeuronCores, gather. Returns (full_out, results)."""
    from concourse.bass_utils import run_bass_kernel_spmd

    x = np.ascontiguousarray(np.asarray(inputs_c_e, np.float32))
    n = x.shape[0]
    assert n == N_TOTAL
    wt = make_wt(Ws, bs).astype(NPDT)
    nc = _get_nc(R_CORE)
    in_maps = [
        {"xt": make_xt(x[c * R_CORE : (c + 1) * R_CORE]), "wt": wt}
        for c in range(N_CORES)
    ]
    res = run_bass_kernel_spmd(nc, in_maps, list(range(N_CORES)), trace=trace, **kw)
    full = np.empty((len(SCALES) * n, DIM), np.float32)
    for si in range(len(SCALES)):
        for c in range(N_CORES):
            full[si * n + c * R_CORE : si * n + (c + 1) * R_CORE] = res.results[c][
                "out"
            ][si * R_CORE : (si + 1) * R_CORE]
    return full, res


def kernel(inputs_c_e, Ws, bs):
    full, _ = run_cores(inputs_c_e, Ws, bs)
    return full


# revision 29
# speedup vs baseline: 1.6997x; 1.0677x over previous
"""Trainium2 Bass kernel for ContractExpand (segment_reduce).

For each scale r in (1,2,4,10,25): segment-sum groups of r consecutive rows,
relu(Linear_r)/r, broadcast back to rows, concat all scales along rows.

Strategy: pure data parallel over 8 NeuronCores (row-sharded, 12500 rows per
core). The host passes x TRANSPOSED ([301, 12500] fp16, row 300 = ones), so
the kernel's input loads are large contiguous DMAs and the contraction dim
is already on partitions.

Per core, per superchunk of up to 3200 rows (matmul data in fp16):
  load:   xT k-slices [128|128|45, sc] straight from DRAM.
  reduce: DVE strided reduce_sum builds r2/r4/r10/r25 segment sums from xT
          (r4, r10 reuse the r2 sums). The ones row reduces to r, which the
          host-side bias row b/r^2 compensates (bias folds into the GEMM).
  mm:     h[g, :] = relu(lhsT.T @ WtExt_r), lhsT = xT (r=1) or a reduced
          region; 3 accumulating k-slices (128+128+45 incl bias), N=300.
  store:  relu result is replicated rep_r times along SBUF free dim, then
          DMA'd with a step-0 broadcast AP -> fully contiguous HBM writes
          with rep_r*1200B descriptors.
"""

import sys

import numpy as np

if "/opt/trn_rl_repo" not in sys.path:
    sys.path.insert(0, "/opt/trn_rl_repo")

from contextlib import ExitStack

import concourse.tile as tile
from concourse import bacc, mybir

DIM = 300
KEXT = 301  # 300 dims + ones row
SCALES = (1, 2, 4, 10, 25)
REP = {1: 1, 2: 2, 4: 4, 10: 5, 25: 5}  # SBUF replication factor per scale
N_TOTAL = 100000
N_CORES = 8
R_CORE = N_TOTAL // N_CORES  # 12500
SC_ROWS = 3200
KSLICES = [(0, 128), (128, 256), (256, 301)]  # sizes 128, 128, 45
F32 = mybir.dt.float32
DT = mybir.dt.float16
NPDT = np.float16
AXX = mybir.AxisListType.X


def _superchunks(rows):
    out = []
    while rows > 0:
        sc = min(SC_ROWS, rows)
        assert sc % 100 == 0
        out.append(sc)
        rows -= sc
    return out


def _emit(ctx, tc, xt_ap, wt_ap, out_ap, rows):
    nc = tc.nc

    singles = ctx.enter_context(tc.tile_pool(name="singles", bufs=1))
    hpool = ctx.enter_context(tc.tile_pool(name="h", bufs=6))
    p2pool = ctx.enter_context(tc.tile_pool(name="p2", bufs=6, space="PSUM"))

    wt_sb = []  # [scale][kslice] -> SBUF tile [ksz, 300] fp16
    for i in range(len(SCALES)):
        per_s = []
        for s, (k0, k1) in enumerate(KSLICES):
            t = singles.tile([k1 - k0, DIM], DT, tag=f"wt{i}_{s}")
            nc.sync.dma_start(out=t[:], in_=wt_ap[i, k0:k1, :])
            per_s.append(t)
        wt_sb.append(per_s)

    scs = _superchunks(rows)
    max_red = sum((SC_ROWS // r + 63) & ~63 for r in SCALES[1:])  # 2880

    # double-buffered xT (k-sliced transposed x) and tmpT (reduced sums)
    xT = [
        [
            singles.tile([k1 - k0, SC_ROWS], DT, tag=f"xT{b}_{s}", name=f"xT{b}_{s}")
            for s, (k0, k1) in enumerate(KSLICES)
        ]
        for b in range(2)
    ]
    tmpT = [
        [
            singles.tile([k1 - k0, max_red], DT, tag=f"tm{b}_{s}", name=f"tm{b}_{s}")
            for s, (k0, k1) in enumerate(KSLICES)
        ]
        for b in range(2)
    ]

    row0 = 0
    for sci, sc_rows in enumerate(scs):
        xb = xT[sci % 2]
        tb = tmpT[sci % 2]
        widths = [sc_rows // r for r in SCALES]
        # region bases padded to 64 fp16 elements (128B) so every matmul
        # lhsT base stays 4-byte aligned even for non-3200 superchunks
        rb, off = [], 0
        for w in widths[1:]:
            rb.append(off)
            off += (w + 63) & ~63

        # --- load xT k-slices (contiguous DMA from host-transposed x) ---
        for s, (k0, k1) in enumerate(KSLICES):
            nc.sync.dma_start(
                out=xb[s][: k1 - k0, :sc_rows],
                in_=xt_ap[k0:k1, row0 : row0 + sc_rows],
            )

        # --- DVE segment reductions (fp16). ones row reduces to r. ---
        lp = nc.allow_low_precision(reason="fp16 segment sums feed fp16 matmul")
        lp.__enter__()
        for s, (k0, k1) in enumerate(KSLICES):
            ksz = k1 - k0
            src = xb[s][:ksz, :sc_rows]
            r2 = tb[s][:ksz, rb[0] : rb[0] + widths[1]]
            nc.vector.reduce_sum(
                out=r2, in_=src.rearrange("p (g r) -> p g r", r=2), axis=AXX
            )
            nc.vector.reduce_sum(
                out=tb[s][:ksz, rb[1] : rb[1] + widths[2]],
                in_=r2.rearrange("p (g r) -> p g r", r=2),
                axis=AXX,
            )
            nc.vector.reduce_sum(
                out=tb[s][:ksz, rb[2] : rb[2] + widths[3]],
                in_=r2.rearrange("p (g r) -> p g r", r=5),
                axis=AXX,
            )
            nc.vector.reduce_sum(
                out=tb[s][:ksz, rb[3] : rb[3] + widths[4]],
                in_=src.rearrange("p (g r) -> p g r", r=25),
                axis=AXX,
            )
        lp.__exit__(None, None, None)

        # --- mm + relu + replicate + expand-store per M tile ---
        for si, r in enumerate(SCALES):
            width = sc_rows // r
            base = 0 if r == 1 else rb[si - 1]
            srcs = xb if r == 1 else tb
            rep = REP[r]
            for c0 in range(0, width, 128):
                M = min(128, width - c0)
                g0 = row0 // r + c0
                psum2 = p2pool.tile([128, DIM], F32, tag="p2")
                for s, (k0, k1) in enumerate(KSLICES):
                    ksz = k1 - k0
                    nc.tensor.matmul(
                        psum2[:M, :],
                        srcs[s][:ksz, base + c0 : base + c0 + M],
                        wt_sb[si][s][:],
                        start=(s == 0),
                        stop=(s == 2),
                    )
                h = hpool.tile([128, 5 * DIM], F32, tag="h")
                nc.scalar.activation(
                    out=h[:M, :DIM],
                    in_=psum2[:M, :],
                    func=mybir.ActivationFunctionType.Relu,
                )
                # replicate along free dim by doubling copies
                done = 1
                while done < rep:
                    cnt = min(done, rep - done)
                    nc.vector.tensor_copy(
                        out=h[:M, done * DIM : (done + cnt) * DIM],
                        in_=h[:M, : cnt * DIM],
                    )
                    done += cnt
                orow = si * rows + g0 * r
                if r == 1:
                    nc.sync.dma_start(
                        out=out_ap[orow : orow + M, :], in_=h[:M, :DIM]
                    )
                else:
                    j = r // rep  # outer broadcast count
                    dst = out_ap[orow : orow + M * r, :].rearrange(
                        "(g j e) d -> g j (e d)", j=j, e=rep
                    )
                    src_b = (
                        h[:M, : rep * DIM]
                        .unsqueeze(1)
                        .broadcast_to([M, j, rep * DIM])
                    )
                    nc.sync.dma_start(out=dst, in_=src_b)

        row0 += sc_rows


def build_nc(rows=R_CORE):
    nc = bacc.Bacc("TRN2", target_bir_lowering=False)
    xt = nc.declare_dram_parameter("xt", [KEXT, rows], DT, isOutput=False)
    wt = nc.declare_dram_parameter(
        "wt", [len(SCALES), KEXT, DIM], DT, isOutput=False
    )
    out = nc.declare_dram_parameter(
        "out", [len(SCALES) * rows, DIM], F32, isOutput=True
    )
    with tile.TileContext(nc) as tc:
        with ExitStack() as ctx:
            _emit(ctx, tc, xt.ap(), wt.ap(), out.ap(), rows)
    nc.compile()
    return nc


def make_wt(Ws, bs):
    """[5, 301, 300]: [W_r.T / r ; b_r / r^2] (ones row reduces to r)."""
    wt = np.empty((len(SCALES), KEXT, DIM), np.float32)
    for i, r in enumerate(SCALES):
        wt[i, :DIM, :] = np.asarray(Ws[i], np.float32).T / r
        wt[i, DIM, :] = np.asarray(bs[i], np.float32) / (r * r)
    return wt


def make_xt(x_shard):
    """[n, 300] fp32 -> [301, n] fp16 with ones in row 300."""
    n = len(x_shard)
    xt = np.empty((KEXT, n), NPDT)
    xt[:DIM, :] = x_shard.astype(NPDT).T
    xt[DIM, :] = 1.0
    return np.ascontiguousarray(xt)


_NC_CACHE = {}


def _get_nc(rows):
    if rows not in _NC_CACHE:
        _NC_CACHE[rows] = build_nc(rows)
    return _NC_CACHE[rows]


def run_cores(inputs_c_e, Ws, bs, trace=False, **kw):
    """Shard, run on the 8 NeuronCores, gather. Returns (full_out, results)."""
    from concourse.bass_utils import run_bass_kernel_spmd

    x = np.ascontiguousarray(np.asarray(inputs_c_e, np.float32))
    n = x.shape[0]
    assert n == N_TOTAL
    wt = make_wt(Ws, bs).astype(NPDT)
    nc = _get_nc(R_CORE)
    in_maps = [
        {"xt": make_xt(x[c * R_CORE : (c + 1) * R_CORE]), "wt": wt}
        for c in range(N_CORES)
    ]
    res = run_bass_kernel_spmd(nc, in_maps, list(range(N_CORES)), trace=trace, **kw)
    full = np.empty((len(SCALES) * n, DIM), np.float32)
    for si in range(len(SCALES)):
        for c in range(N_CORES):
            full[si * n + c * R_CORE : si * n + (c + 1) * R_CORE] = res.results[c][
                "out"
            ][si * R_CORE : (si + 1) * R_CORE]
    return full, res


def kernel(inputs_c_e, Ws, bs):
    full, _ = run_cores(inputs_c_e, Ws, bs)
    return full
